# revision 10
# baseline (speedup 1.0000x reference)
"""Trainium2 Bass kernel for HET_EglRelGraphConv (relational GCN, basis decomposition).

Full inputs -> full output. Internally shards destination-node windows across
8 NeuronCores; each core computes its output rows exactly (no collectives).

Per relation r:  out += (S_r @ A_r) @ W_r
  A_r: gathered src features for r's edges (dma_gather), scaled by edge norm
       (folded into the one-hot instead)
  S_r: one-hot [dst, edge] scatter matrix; S_r @ A_r computed as a PE matmul
       with lhsT = A_chunk (edges on K) and rhs = onehot*norm  -> B^T in PSUM
  W_r = sum_b w_comp[r,b] * bases[b]  (precomputed)
All matmuls run in float32r (TF32-like, ~1.4e-4 rel err); accumulation fp32.
"""
import os
import sys
import types

sys.path.insert(0, "/opt/trn_rl_repo")

import numpy as np


def _install_ntff_hook():
    # Minimal antenv.axon_hooks so BASS_TRACE=1 can capture NTFF under axon.
    try:
        import antenv
        try:
            from antenv.axon_hooks import set_axon_ntff_profile_hook  # noqa
            have = True
        except ImportError:
            have = False
        if not have:
            m = types.ModuleType("antenv.axon_hooks")
            m._hook = None

            def _set(h):
                m._hook = h

            def _get():
                return m._hook

            m.set_axon_ntff_profile_hook = _set
            m.get_axon_ntff_profile_hook = _get
            sys.modules["antenv.axon_hooks"] = m
            antenv.axon_hooks = m
        from antenv.axon_hooks import set_axon_ntff_profile_hook as _sh
        from trn_agent_boot.trn_boot import _ntff_profile_via_ctypes
        _sh(_ntff_profile_via_ctypes("/opt/axon/libaxon_pjrt.so"))
    except Exception:
        pass


_install_ntff_hook()

import concourse.bacc as bacc
import concourse.mybir as mybir
import concourse.tile as tile
from concourse import bass_utils

N_NODES = 20000
N_EDGES = 320000
F = 512          # in/out feature dim
R = 32           # relations
NB = 8           # bases
WIN = 256        # dst rows per window
CORES = 8
NWIN = 80        # 80 * 256 = 20480 padded rows
WPC = NWIN // CORES          # windows per core = 10
ROWS_PC = WPC * WIN          # 2560 output rows per core
PASSES = 4
RPP = R // PASSES            # relations per pass = 8
CHUNK = 128

f32 = mybir.dt.float32
f32r = mybir.dt.float32r
i16 = mybir.dt.int16

LAST_RESULTS = None


def _prep(x, bases, w_comp, h_bias, norm, src, dst, etype):
    src = np.asarray(src).astype(np.int64)
    dst = np.asarray(dst).astype(np.int64)
    etype = np.asarray(etype).astype(np.int64)
    norm = np.asarray(norm, dtype=np.float32).reshape(-1)
    x = np.asarray(x, dtype=np.float32)
    bases = np.asarray(bases, dtype=np.float32)
    w_comp = np.asarray(w_comp, dtype=np.float32)
    h_bias = np.asarray(h_bias, dtype=np.float32)

    W = np.einsum("rb,bio->rio", w_comp, bases).astype(np.float32)  # [R, F, F]
    # device layout: [PASSES, RPP, 4, 128, F]
    W_dev = np.ascontiguousarray(
        W.reshape(PASSES, RPP, 4, CHUNK, F)
    )

    win = dst // WIN                      # 0..79 (dst < 20000 -> win <= 78)
    core = win // WPC
    lw = win % WPC
    p = etype // RPP
    j = etype % RPP

    # group linear id within a core: ((p*WPC + lw)*RPP + j)
    gid = (p * WPC + lw) * RPP + j
    NG = PASSES * WPC * RPP               # groups per core = 320

    # counts per (core, group)
    n = np.zeros((CORES, NG), np.int64)
    np.add.at(n, (core, gid), 1)
    C_flat = np.maximum(0, -(-n.max(axis=0) // CHUNK))          # [NG] chunks per group
    padded = C_flat * CHUNK
    goff = np.zeros(NG + 1, np.int64)
    np.cumsum(padded, out=goff[1:])
    TOTPAD = int(goff[-1])
    TOTCH = TOTPAD // CHUNK

    # chunk-column offset of each group
    choff = goff // CHUNK

    # per-core padded arrays
    idx_all = []
    dn_all = []
    nm_all = []
    for k in range(CORES):
        m = core == k
        g = gid[m]
        order = np.argsort(g, kind="stable")
        g_sorted = g[order]
        # rank within group
        starts = np.searchsorted(g_sorted, np.arange(NG))
        rank = np.arange(g_sorted.size) - starts[g_sorted]
        pos = goff[g_sorted] + rank
        idx = np.zeros(TOTPAD, np.int16)
        dnf = np.zeros(TOTPAD, np.float32)
        nmf = np.zeros(TOTPAD, np.float32)
        idx[pos] = src[m][order].astype(np.int16)
        dnf[pos] = (dst[m][order] % WIN).astype(np.float32)
        nmf[pos] = norm[m][order]
        idx_all.append(idx)
        dn_all.append(dnf)
        nm_all.append(nmf)

    # gather blocks: per (p, lw, half): rels j in [4*half, 4*half+4)
    # block chunk range = [choff[first group], choff[last group + 4])
    gathers = []  # (chunk_start, nch) in group order
    for pp in range(PASSES):
        for w in range(WPC):
            for half in range(2):
                g0 = (pp * WPC + w) * RPP + 4 * half
                g1 = g0 + 4
                c0 = int(choff[g0])
                c1 = int(choff[g1]) if g1 < NG else TOTCH
                gathers.append((c0, c1 - c0))
    GMAX = max(nch for _, nch in gathers) if gathers else 1
    GMAX = max(GMAX, 1)

    # idx wrapped per gather block: [128, TOTPAD//16] int16
    def wrap_idx(idx):
        out = np.zeros((128, TOTPAD // 16), np.int16)
        for c0, nch in gathers:
            if nch == 0:
                continue
            blk = idx[c0 * CHUNK:(c0 + nch) * CHUNK]
            wb = blk.reshape(-1, 16).T          # [16, len/16]
            col0 = c0 * CHUNK // 16
            for rep in range(8):
                out[rep * 16:(rep + 1) * 16, col0:col0 + wb.shape[1]] = wb
        return out

    idx_wrapped = [wrap_idx(ia) for ia in idx_all]
    # dstoff / norm by chunk column: [128, TOTCH]
    dn_cols = [np.ascontiguousarray(d.reshape(TOTCH, CHUNK).T) for d in dn_all]
    nm_cols = [np.ascontiguousarray(d.reshape(TOTCH, CHUNK).T) for d in nm_all]

    iota = np.tile(np.arange(WIN, dtype=np.float32), (128, 1))
    bias2 = np.tile(h_bias, (128, 2)).astype(np.float32)        # [128, 2*F]

    meta = dict(
        C=C_flat.reshape(PASSES, WPC, RPP),
        choff=choff[:-1].reshape(PASSES, WPC, RPP),
        gathers=gathers,
        GMAX=GMAX,
        TOTCH=TOTCH,
        TOTPAD=TOTPAD,
    )
    return x, W_dev, idx_wrapped, dn_cols, nm_cols, iota, bias2, meta


def _build_program(meta):
    C = meta["C"]
    choff = meta["choff"]
    gathers = meta["gathers"]
    GMAX = meta["GMAX"]
    TOTCH = max(meta["TOTCH"], 1)
    TOTPAD = meta["TOTPAD"]

    nc = bacc.Bacc("TRN2", target_bir_lowering=False, debug=False,
                   num_devices=CORES)
    x_t = nc.dram_tensor("x", [N_NODES, F], f32r, kind="ExternalInput").ap()
    w_t = nc.dram_tensor("w", [PASSES, RPP, 4, CHUNK, F], f32r,
                         kind="ExternalInput").ap()
    idx_t = nc.dram_tensor("idx", [128, max(TOTPAD // 16, 1)], i16,
                           kind="ExternalInput").ap()
    dn_t = nc.dram_tensor("dn", [128, TOTCH], f32, kind="ExternalInput").ap()
    nm_t = nc.dram_tensor("nm", [128, TOTCH], f32, kind="ExternalInput").ap()
    iota_t = nc.dram_tensor("iota", [128, WIN], f32, kind="ExternalInput").ap()
    bias_t = nc.dram_tensor("bias", [128, 2 * F], f32, kind="ExternalInput").ap()
    out_t = nc.dram_tensor("out", [ROWS_PC, F], f32, kind="ExternalOutput").ap()
    part_t = nc.dram_tensor("part", [WPC, 128, 2, F], f32, kind="Internal").ap()
    KDBG = bool(os.environ.get("KDBG"))
    if KDBG:
        dbg_gt_t = nc.dram_tensor("dbg_gt", [128, GMAX, F], f32,
                                  kind="ExternalOutput").ap()
        dbg_bts_t = nc.dram_tensor("dbg_bts", [RPP, 128, 4 * WIN], f32,
                                   kind="ExternalOutput").ap()
        dbg_st_t = nc.dram_tensor("dbg_st", [128, 2 * F], f32,
                                  kind="ExternalOutput").ap()
        dbg_oh_t = nc.dram_tensor("dbg_oh", [128, WIN], f32,
                                  kind="ExternalOutput").ap()

    out_v = out_t.rearrange("(lw m p) o -> lw p m o", m=2, p=128)  # [WPC,128,2,F]

    with tile.TileContext(nc) as tc:
        with (
            tc.tile_pool(name="wp", bufs=1) as wp,
            tc.tile_pool(name="gat", bufs=2) as gat,
            tc.tile_pool(name="idxp", bufs=2) as idxp,
            tc.tile_pool(name="btp", bufs=2, space="PSUM") as btpp,
            tc.tile_pool(name="outp", bufs=2, space="PSUM") as outpp,
            tc.tile_pool(name="bts", bufs=9) as btsp,
            tc.tile_pool(name="oh", bufs=int(C.max()) + 3) as ohp,
            tc.tile_pool(name="pt", bufs=2) as ptp,
            tc.tile_pool(name="st", bufs=3) as stp,
            tc.tile_pool(name="cst", bufs=1) as cst,
        ):
            dn = cst.tile([128, TOTCH], f32, tag="dn")
            nm = cst.tile([128, TOTCH], f32, tag="nm")
            iota = cst.tile([128, WIN], f32, tag="iota")
            bias = cst.tile([128, 2 * F], f32, tag="bias")
            nc.sync.dma_start(dn[:], dn_t[:])
            nc.sync.dma_start(nm[:], nm_t[:])
            nc.sync.dma_start(iota[:], iota_t[:])
            nc.sync.dma_start(bias[:], bias_t[:])

            for p in range(PASSES):
                wt = wp.tile([128, RPP * 4 * F], f32r, tag="w")
                nc.sync.dma_start(
                    wt[:].rearrange("q (j ib o) -> q j ib o", j=RPP, ib=4),
                    w_t[p].rearrange("j ib q o -> q j ib o"),
                )
                for lw in range(WPC):
                    gi = (p * WPC + lw) * 2
                    gts = []
                    for half in range(2):
                        c0, nch = gathers[gi + half]
                        if nch == 0:
                            gts.append((None, c0))
                            continue
                        it = idxp.tile([128, GMAX * 8], i16, tag="idx")
                        nc.sync.dma_start(
                            it[:, :nch * 8],
                            idx_t[:, c0 * 8:(c0 + nch) * 8],
                        )
                        gt = gat.tile([128, GMAX, F], f32r, tag="gat")
                        nc.gpsimd.dma_gather(
                            out_ap=gt[:, :nch, :],
                            in_ap=x_t[:],
                            idxs_ap=it[:, :nch * 8],
                            num_idxs=nch * CHUNK,
                            num_idxs_reg=nch * CHUNK,
                            elem_size=F,
                        )
                        if KDBG and p == 0 and lw == 0 and half == 0:
                            nc.sync.dma_start(
                                dbg_gt_t[:, :nch, :],
                                gt[:, :nch, :].bitcast(f32),
                            )
                        gts.append((gt, c0))

                    if p > 0:
                        pt = ptp.tile([128, 2, F], f32, tag="pt")
                        nc.sync.dma_start(pt[:], part_t[lw])

                    bts_list = []
                    for j in range(RPP):
                        cj = int(C[p, lw, j])
                        if cj == 0:
                            bts_list.append(None)
                            continue
                        gt, gc0 = gts[j // 4]
                        c0 = int(choff[p, lw, j])
                        btps = btpp.tile([128, 4 * WIN], f32, tag="btp")
                        ohs = []
                        for c in range(cj):
                            col = c0 + c
                            oh = ohp.tile([128, WIN], f32r, tag="oh")
                            nc.vector.tensor_scalar(
                                oh[:], iota[:],
                                dn[:, col:col + 1], nm[:, col:col + 1],
                                op0=mybir.AluOpType.is_equal,
                                op1=mybir.AluOpType.mult,
                            )
                            if KDBG and p == 0 and lw == 0 and j == 0 and c == 0:
                                nc.sync.dma_start(dbg_oh_t[:],
                                                  oh[:].bitcast(f32))
                            ohs.append(oh)
                        # each ib region's accumulation group must complete
                        # before the next starts: start=True clears has_written
                        # bank-wide, so interleaving groups within a bank
                        # corrupts accumulation.
                        for ib in range(4):
                            for c in range(cj):
                                lc = c0 + c - gc0
                                nc.tensor.matmul(
                                    btps[:, ib * WIN:(ib + 1) * WIN],
                                    gt[:, lc, ib * 128:(ib + 1) * 128],
                                    ohs[c][:],
                                    start=(c == 0), stop=(c == cj - 1),
                                )
                        bts = btsp.tile([128, 4 * WIN], f32r, tag="bts")
                        nc.scalar.copy(bts[:], btps[:])
                        if KDBG and p == 0 and lw == 0:
                            nc.sync.dma_start(dbg_bts_t[j],
                                              bts[:].bitcast(f32))
                        bts_list.append(bts)

                    mmlist = [(j, ib) for j in range(RPP)
                              if bts_list[j] is not None for ib in range(4)]
                    op_ps = outpp.tile([128, 2 * F], f32, tag="outp")
                    for m in range(2):
                        for q, (j, ib) in enumerate(mmlist):
                            nc.tensor.matmul(
                                op_ps[:, m * F:(m + 1) * F],
                                bts_list[j][:, ib * WIN + m * 128:
                                            ib * WIN + (m + 1) * 128],
                                wt[:, (j * 4 + ib) * F:(j * 4 + ib + 1) * F],
                                start=(q == 0), stop=(q == len(mmlist) - 1),
                            )

                    st = stp.tile([128, 2 * F], f32, tag="st")
                    opv = op_ps[:].rearrange("q (m o) -> q m o", m=2)
                    if p == 0:
                        nc.vector.tensor_copy(st[:], op_ps[:])
                    else:
                        nc.vector.tensor_tensor(
                            st[:], op_ps[:],
                            pt[:].rearrange("q m o -> q (m o)"),
                            op=mybir.AluOpType.add,
                        )
                    if KDBG and p == 0 and lw == 0:
                        nc.sync.dma_start(dbg_st_t[:], st[:])
                    if p < PASSES - 1:
                        nc.sync.dma_start(
                            part_t[lw],
                            st[:].rearrange("q (m o) -> q m o", m=2),
                        )
                    else:
                        st2 = stp.tile([128, 2 * F], f32, tag="st2")
                        nc.vector.tensor_tensor(
                            st2[:], st[:], bias[:], op=mybir.AluOpType.add,
                        )
                        nc.sync.dma_start(
                            out_v[lw],
                            st2[:].rearrange("q (m o) -> q m o", m=2),
                        )

    nc.compile()
    return nc


def kernel(x, bases, w_comp, h_bias, norm, src, dst, etype):
    global LAST_RESULTS
    xf, W_dev, idx_w, dn_c, nm_c, iota, bias2, meta = _prep(
        x, bases, w_comp, h_bias, norm, src, dst, etype)
    nc = _build_program(meta)
    in_maps = []
    for k in range(CORES):
        in_maps.append({
            "x": xf,
            "w": W_dev,
            "idx": idx_w[k],
            "dn": dn_c[k],
            "nm": nm_c[k],
            "iota": iota,
            "bias": bias2,
        })
    res = bass_utils.run_bass_kernel_spmd(
        nc, in_maps, core_ids=list(range(CORES)),
        trace=bool(os.environ.get("BASS_TRACE")),
    )
    LAST_RESULTS = res
    out = np.concatenate([res.results[k]["out"] for k in range(CORES)], axis=0)
    return np.ascontiguousarray(out[:N_NODES])
